# revision 20
# baseline (speedup 1.0000x reference)
"""ChessNNUE Trainium2 kernel — sparse K-compaction version.

Strategy (data-parallel over batch, 8 NeuronCores):
  - Each core handles 512 of the 4096 batch rows, split into 4 batch
    tiles of 128 rows.
  - The features are ~0.07% dense (~30 active of 40960 per row).  For
    each (batch-tile, perspective) the host computes the union of
    active feature columns (~3.7k expected for 128 rows), gathers just
    those ft_w rows into a compact [KTP*128, 1024] bf16 block, and
    builds the matching compact feature block [KTP*128, 128].  One
    extra K-row carries ft_b with feature value 1.0, so the bias falls
    out of the matmul for free.
  - The device then runs a dense bf16 matmul over the compact K
    (~32 K-tiles instead of 320), accumulating fp32 in PSUM across the
    whole K range per (persp, hid-chunk).  PE work and HBM traffic
    both drop ~9x vs the dense kernel.
  - Host pre-arranges every K-group as its own fully contiguous
    [128 part, gsz*...] dram block; weight DMAs alternate between the
    two HWDGE rings (sync/scalar) to overlap per-DMA turnaround.
  - stm select/clip + the small MLP head run on-chip per batch tile
    (head lagged one tile so the PE queue never stalls on the DVE
    blend chain); the device returns raw [1, 512] per core and the
    sigmoid is applied on host.
"""

import math
import numpy as np
from contextlib import ExitStack

import concourse.bass as bass
import concourse.tile as tile
from concourse import bacc, mybir
from concourse.bass_utils import run_bass_kernel_spmd

B, FEAT, HID = 4096, 40960, 1024
L1, L2 = 64, 32
NCORES = 8
BC = B // NCORES          # 512 batch rows per core
NT = 4                    # batch tiles per core
TN = BC // NT             # 128 rows per batch tile
NHC = HID // 128          # 8 hid chunks of 128
KTP_DEFAULT = 32          # compact K tiles (of 128) incl. bias row
GK_DEFAULT = 16           # K tiles per DMA group

F32 = mybir.dt.float32
BF16 = mybir.dt.bfloat16
FP8 = mybir.dt.float8e4

_CACHE = {}

try:
    from ml_dtypes import bfloat16 as np_bf16
    from ml_dtypes import float8_e4m3 as np_fp8
except ImportError:  # pragma: no cover
    import jax.numpy as _jnp
    np_bf16 = _jnp.bfloat16
    np_fp8 = _jnp.float8_e4m3


def _build(kt_pad=KTP_DEFAULT, gk=GK_DEFAULT, reps=1, mode="full"):
    """Build + compile the per-core Bass program. Returns nc.

    mode: "full" | "dmaonly" (main DMAs, no compute) | "nodma"
          (compute on static SBUF garbage).
    """
    groups = []
    rem = kt_pad
    while rem > 0:
        groups.append(min(gk, rem))
        rem -= min(gk, rem)
    ng = len(groups)
    Alu = mybir.AluOpType
    Act = mybir.ActivationFunctionType

    nc = bacc.Bacc("TRN2", target_bir_lowering=False, debug=False,
                   num_devices=NCORES)

    # compact weights / features, one dram tensor per K-group so every
    # main-loop DMA is a fully contiguous [128, gsz*...] row-block
    cws = [nc.dram_tensor(f"cw{g}", (NT * 2 * 128, groups[g] * HID), BF16,
                          kind="ExternalInput") for g in range(ng)]
    cfs = [nc.dram_tensor(f"cf{g}", (NT * 2 * 128, groups[g] * TN), FP8,
                          kind="ExternalInput") for g in range(ng)]
    stm = nc.dram_tensor("stm", (BC,), F32, kind="ExternalInput")
    l1w = nc.dram_tensor("l1w", (2 * HID, L1), F32, kind="ExternalInput")
    l1b = nc.dram_tensor("l1b", (L1,), F32, kind="ExternalInput")
    l2w = nc.dram_tensor("l2w", (L1, L2), F32, kind="ExternalInput")
    l2b = nc.dram_tensor("l2b", (L2,), F32, kind="ExternalInput")
    l3w = nc.dram_tensor("l3w", (L2, 1), F32, kind="ExternalInput")
    l3b = nc.dram_tensor("l3b", (1,), F32, kind="ExternalInput")
    out = nc.dram_tensor("out", (1, BC), F32, kind="ExternalOutput")

    with ExitStack() as ctx:
        tc = ctx.enter_context(tile.TileContext(nc))
        const = ctx.enter_context(tc.tile_pool(name="const", bufs=1))
        wpool = ctx.enter_context(tc.tile_pool(name="wpool", bufs=2))
        fpool = ctx.enter_context(tc.tile_pool(name="fpool", bufs=2))
        h1pool = ctx.enter_context(tc.tile_pool(name="h1pool", bufs=2))
        accpool = ctx.enter_context(tc.tile_pool(name="accpool", bufs=1))
        tmppool = ctx.enter_context(tc.tile_pool(name="tmppool", bufs=2))
        outpool = ctx.enter_context(tc.tile_pool(name="outpool", bufs=2))
        psum = ctx.enter_context(
            tc.tile_pool(name="psum", bufs=6, space="PSUM"))
        psumh = ctx.enter_context(
            tc.tile_pool(name="psumh", bufs=2, space="PSUM"))

        # ---------- constants (loaded once, excluded from rep loop) ----------
        l1b_sb = const.tile([L1, 1], F32)
        nc.sync.dma_start(l1b_sb[:], l1b.ap())
        l2b_sb = const.tile([L2, 1], F32)
        nc.sync.dma_start(l2b_sb[:], l2b.ap())
        l3b_sb = const.tile([1, 1], F32)
        nc.sync.dma_start(l3b_sb[:], l3b.ap())

        l1w_sb = const.tile([128, (2 * HID) // 128, L1], BF16)
        nc.gpsimd.dma_start(l1w_sb[:],
                            l1w.ap().rearrange("(t p) m -> p t m", p=128))
        l2w_sb = const.tile([L1, L2], BF16)
        nc.gpsimd.dma_start(l2w_sb[:], l2w.ap())
        l3w_sb = const.tile([L2, 1], BF16)
        nc.gpsimd.dma_start(l3w_sb[:], l3w.ap())

        stm_bf = const.tile([1, BC], BF16)
        nc.gpsimd.dma_start(stm_bf[:], stm.ap())
        ones_bf = const.tile([1, 128], BF16)
        nc.vector.memset(ones_bf[:], 1.0)

        # broadcast stm across partitions: [128, BC] = ones[1,128].T @ stm[1,BC]
        ps_stm = psumh.tile([128, BC], F32, tag="hd")
        nc.tensor.matmul(ps_stm[:], ones_bf[:], stm_bf[:],
                         start=True, stop=True)
        stmb_sb = const.tile([128, BC], F32)
        nc.vector.tensor_copy(stmb_sb[:], ps_stm[:])

        do_dma = mode in ("full", "dmaonly")
        do_pe = mode in ("full", "nodma")
        if mode == "null":
            for _rep in range(reps):
                raw_sb = outpool.tile([1, BC], F32, tag="raw")
                nc.vector.memset(raw_sb[:], 0.0)
                nc.sync.dma_start(out.ap()[0:1, :], raw_sb[:])

        if mode == "nodma":
            wt_s = [wpool.tile([128, gk * HID], BF16, tag=f"wt{s}",
                               name=f"wts{s}") for s in range(2)]
            ft_s = [fpool.tile([128, gk * TN], BF16, tag=f"ft{s}",
                               name=f"fts{s}") for s in range(2)]
            for s in range(2):
                nc.vector.memset(wt_s[s][:, 0:8], 0.0)
                nc.vector.memset(ft_s[s][:, 0:8], 0.0)

        def emit_tile(ti, raw_sb):
            """Chains + evac + blend + clips for one batch tile.  Returns a
            head context to be emitted later (head lags one tile so the PE
            queue never stalls on the DVE blend chain)."""
            if True:
                n0 = ti * TN
                # persistent fp32 accumulators: [0..7] white, [8..15] black
                accs = [accpool.tile([128, TN], F32, tag=f"acc{i}",
                                     name=f"acc{i}") for i in range(2 * NHC)]
                for g in range(ng):
                    gsz = groups[g]
                    weng = nc.sync if g % 2 == 0 else nc.scalar
                    wts, fts = [None, None], [None, None]
                    for s in range(2):
                        if do_dma:
                            r0 = (ti * 2 + s) * 128
                            wt = wpool.tile([128, gk * HID], BF16,
                                            tag=f"wt{s}", name=f"wt{s}")
                            weng.dma_start(
                                wt[:, 0:gsz * HID],
                                cws[g].ap()[r0:r0 + 128, :])
                            ft = fpool.tile([128, gk * TN], BF16,
                                            tag=f"ft{s}", name=f"ft{s}")
                            nc.gpsimd.dma_start(
                                ft[:, 0:gsz * TN],
                                cfs[g].ap()[r0:r0 + 128, :])
                        else:
                            wt, ft = wt_s[s], ft_s[s]
                        wts[s], fts[s] = wt, ft

                    if do_pe:
                        for s in range(2):
                            for c in range(NHC):
                                pst = psum.tile([128, 512], F32, tag="mm")
                                wt, ft = wts[s], fts[s]
                                for t in range(gsz):
                                    nc.tensor.matmul(
                                        pst[:, 0:TN],
                                        wt[:, t * HID + c * 128:
                                           t * HID + (c + 1) * 128],
                                        ft[:, t * TN:(t + 1) * TN],
                                        start=(t == 0), stop=(t == gsz - 1))
                                a = accs[s * NHC + c]
                                if g == 0:
                                    nc.vector.tensor_copy(a[:], pst[:, 0:TN])
                                else:
                                    nc.vector.tensor_add(a[:], a[:],
                                                         pst[:, 0:TN])
                if not do_pe:
                    for i in range(2 * NHC):
                        nc.vector.memset(accs[i][:, 0:8], 0.0)

                # ---- stm blend + clip -> h1 chunks (bf16) ----
                # top = b + stm*(w-b) ; bot = w - stm*(w-b)   (bias already in)
                h1s = [h1pool.tile([128, TN], BF16, tag=f"h1_{i}",
                                   name=f"h1_{i}") for i in range(2 * NHC)]
                stm_sl = stmb_sb[:, n0:n0 + TN]
                for c in range(NHC):
                    wp = accs[c][:]
                    bp = accs[NHC + c][:]
                    d = tmppool.tile([128, TN], F32, tag="d")
                    nc.vector.tensor_sub(d[:], wp, bp)
                    m = tmppool.tile([128, TN], F32, tag="m")
                    nc.vector.tensor_mul(m[:], d[:], stm_sl)
                    topf = tmppool.tile([128, TN], F32, tag="topf")
                    nc.vector.tensor_add(topf[:], bp, m[:])
                    botf = tmppool.tile([128, TN], F32, tag="botf")
                    nc.vector.tensor_sub(botf[:], wp, m[:])
                    nc.vector.tensor_scalar(
                        h1s[c][:], topf[:], 0.0, 1.0, Alu.max, Alu.min)
                    nc.vector.tensor_scalar(
                        h1s[NHC + c][:], botf[:], 0.0, 1.0, Alu.max, Alu.min)
            return dict(h1s=h1s, n0=n0, ti=ti, raw_sb=raw_sb)

        def emit_head(hc):
            h1s, n0 = hc["h1s"], hc["n0"]
            raw_sb = hc["raw_sb"]
            if True:
                # ---- head ----
                hd = psumh.tile([128, 512], F32, tag="hd")
                for t in range(2 * NHC):
                    nc.tensor.matmul(hd[0:L1, 0:TN], l1w_sb[:, t, :],
                                     h1s[t][:], start=(t == 0),
                                     stop=(t == 2 * NHC - 1))
                h2f = tmppool.tile([L1, TN], F32, tag="h2f")
                nc.vector.tensor_scalar(h2f[:], hd[0:L1, 0:TN], l1b_sb[:],
                                        0.0, Alu.add, Alu.max)
                h2 = tmppool.tile([L1, TN], BF16, tag="h2")
                nc.vector.tensor_scalar(h2[:], h2f[:], 1.0, None, Alu.min)

                nc.tensor.matmul(hd[0:L2, 128:128 + TN], l2w_sb[:], h2[:],
                                 start=True, stop=True)
                h3f = tmppool.tile([L2, TN], F32, tag="h3f")
                nc.vector.tensor_scalar(h3f[:], hd[0:L2, 128:128 + TN],
                                        l2b_sb[:], 0.0, Alu.add, Alu.max)
                h3 = tmppool.tile([L2, TN], BF16, tag="h3")
                nc.vector.tensor_scalar(h3[:], h3f[:], 1.0, None, Alu.min)

                nc.tensor.matmul(hd[0:1, 256:256 + TN], l3w_sb[:], h3[:],
                                 start=True, stop=True)
                nc.vector.tensor_scalar(raw_sb[0:1, n0:n0 + TN],
                                        hd[0:1, 256:256 + TN], l3b_sb[:],
                                        None, Alu.add)
            if hc["ti"] == NT - 1:
                # last tile of its rep: flush that rep's outputs
                nc.sync.dma_start(out.ap()[0:1, :], raw_sb[:])

        if mode != "null":
            pending = None
            for _rep in range(reps):
                raw_sb = outpool.tile([1, BC], F32, tag="raw")
                for ti in range(NT):
                    hc = emit_tile(ti, raw_sb)
                    if pending is not None:
                        emit_head(pending)
                    pending = hc
            emit_head(pending)

    nc.compile()
    return nc


def _get_nc(kt_pad=KTP_DEFAULT, gk=GK_DEFAULT):
    key = (kt_pad, gk)
    if key not in _CACHE:
        _CACHE[key] = _build(kt_pad=kt_pad, gk=gk)
    return _CACHE[key]


def _prep_in_maps(white_features, black_features, stm, ft_w, ft_b,
                  l1_w, l1_b, l2_w, l2_b, l3_w, l3_b,
                  kt_pad=None, gk=GK_DEFAULT):
    """Returns (in_maps, kt_pad)."""
    f32 = lambda a: np.ascontiguousarray(np.asarray(a, dtype=np.float32))
    white = np.asarray(white_features, dtype=np.float32)
    black = np.asarray(black_features, dtype=np.float32)
    feat = white.shape[1]
    hid = ft_w.shape[0]
    stm = np.asarray(stm, dtype=np.float32).reshape(-1)
    wT_bf = np.ascontiguousarray(
        np.asarray(ft_w, dtype=np.float32).T).astype(np_bf16)  # [FEAT, HID]
    ftb_bf = np.asarray(ft_b, dtype=np.float32).astype(np_bf16)
    l1wT = f32(np.asarray(l1_w, dtype=np.float32).T)      # [2048, 64]
    l2wT = f32(np.asarray(l2_w, dtype=np.float32).T)      # [64, 32]
    l3wT = f32(np.asarray(l3_w, dtype=np.float32).T)      # [32, 1]
    l1b, l2b, l3b = f32(l1_b), f32(l2_b), f32(l3_b)

    # active-column unions per (core, tile, persp)
    idx_all = []
    max_u = 0
    for c in range(NCORES):
        for ti in range(NT):
            r0 = c * BC + ti * TN
            for src in (white, black):
                X = src[r0:r0 + TN]
                idx = np.flatnonzero(X.any(axis=0))
                idx_all.append(idx)
                max_u = max(max_u, len(idx))
    kt_needed = (max_u + 1 + 127) // 128
    if kt_pad is None or kt_pad == "tight":
        kt_pad = kt_needed
    assert kt_needed <= kt_pad, (kt_needed, kt_pad)
    K = kt_pad * 128

    groups = []
    rem = kt_pad
    while rem > 0:
        groups.append(min(gk, rem))
        rem -= min(gk, rem)
    goffs = [sum(groups[:g]) for g in range(len(groups))]

    in_maps = []
    it = iter(idx_all)
    for c in range(NCORES):
        cwg = [np.zeros((NT * 2, 128, gs * hid), np_bf16) for gs in groups]
        cfg = [np.zeros((NT * 2, 128, gs * TN), np_fp8) for gs in groups]
        for ti in range(NT):
            r0 = c * BC + ti * TN
            for s, srcm in enumerate((white, black)):
                idx = next(it)
                u = len(idx)
                wblk = np.zeros((K, hid), np_bf16)
                wblk[:u] = wT_bf[idx]
                wblk[u] = ftb_bf                       # bias row
                fblk = np.zeros((K, TN), np_fp8)
                fblk[:u] = srcm[r0:r0 + TN, idx].T
                fblk[u] = 1.0                          # bias activation
                wb = wblk.reshape(kt_pad, 128, hid)
                fb = fblk.reshape(kt_pad, 128, TN)
                for g, gs in enumerate(groups):
                    o = goffs[g]
                    cwg[g][ti * 2 + s] = wb[o:o + gs].transpose(
                        1, 0, 2).reshape(128, -1)
                    cfg[g][ti * 2 + s] = fb[o:o + gs].transpose(
                        1, 0, 2).reshape(128, -1)
        sl = slice(c * BC, (c + 1) * BC)
        im = dict(stm=f32(stm[sl]), l1w=l1wT, l1b=l1b,
                  l2w=l2wT, l2b=l2b, l3w=l3wT, l3b=l3b)
        for g in range(len(groups)):
            im[f"cw{g}"] = cwg[g].reshape(NT * 2 * 128, -1)
            im[f"cf{g}"] = cfg[g].reshape(NT * 2 * 128, -1)
        in_maps.append(im)
    return in_maps, kt_pad


def _assemble(results):
    raw = np.concatenate([results[c]["out"][0] for c in range(NCORES)])
    raw = raw.reshape(B, 1).astype(np.float32)
    sig = (1.0 / (1.0 + np.exp(-raw.astype(np.float64)))).astype(np.float32)
    return sig, raw


def kernel(**inputs):
    in_maps, kt_pad = _prep_in_maps(**inputs)
    nc = _get_nc(kt_pad=kt_pad)
    res = run_bass_kernel_spmd(nc, in_maps, core_ids=list(range(NCORES)))
    return _assemble(res.results)
